# revision 23
# baseline (speedup 1.0000x reference)
"""Trainium2 Bass kernel for nn_BartPooler_53815940219079 (segment_reduce).

Computes, for each of B*T segments of a [B, S, H] hidden-state tensor:
  feat = concat([segment_max, segment_mean])  -> tanh(feat @ W.T + b)

Strategy (8 NeuronCores, SPMD — one program, per-core data):
  * Host compacts each segment's used tokens into a per-core fp16 token
    stream, padding every segment with duplicates of its first token so
    each segment occupies a whole number of G-token "groups" (plus a
    compensation group whose negative membership weight cancels the
    duplicate tokens in the sum).  Segments are dealt snake-wise across
    cores by size so all cores share one static layout.
  * Device, per 128-group tile: grouped max/sum partials (DVE fp16);
    per-segment mean accumulation via membership matmuls on PE (weights
    fold in 1/count, fp32 PSUM); PE transposes of the max partials; per-
    segment max reduce on DVE; then a fused [2H] x [2H, D] fp16 GEMM
    with bias folded in as a rank-1 matmul, tanh on ScalarE.
"""

import numpy as np

import concourse.bacc as bacc
import concourse.mybir as mybir
import concourse.tile as tile
from concourse.bass_utils import run_bass_kernel_spmd
from concourse.masks import make_identity
from concourse.tile import add_dep_helper

NCORES = 8
G = 8          # tokens per group
PTILE = 128 * G  # tokens per main tile

B, S, H, T = 16, 4096, 1024, 16
D_OUT = 1024
HB = H // 128  # h-blocks per hidden vector
KB = 2 * HB    # k-blocks in the feature GEMM

F32 = mybir.dt.float32
F16 = mybir.dt.float16


def _build_schedule(parts, turns):
    """Host-side: segment list -> per-core compacted layout (uniform shapes)."""
    Bn, Tn = parts.shape
    segs = []  # (global_row, example, start_token, count)
    for b in range(Bn):
        cum = 0
        for j in range(Tn):
            c = int(parts[b, j])
            if j < int(turns[b]):
                segs.append((b * Tn + j, b, 1 + cum, c))
            cum += c

    # Deal segments to cores by size rank: slot j holds the 8 segments of
    # ranks [8j, 8j+8), one per core, so the uniform per-slot group count
    # L[j] (max over cores) is as tight as possible.
    order = sorted(range(len(segs)), key=lambda i: -segs[i][3])
    core_slots = [[] for _ in range(NCORES)]
    for rank, i in enumerate(order):
        core_slots[rank % NCORES].append(segs[i])
    seg_cap = max(len(s) for s in core_slots)

    def groups_needed(cnt):
        g = (cnt + G - 1) // G
        if cnt % G:
            g += 1  # at least one pure-duplicate group for the compensation
        return g

    # Uniform per-slot group counts across cores.
    L = []
    for j in range(seg_cap):
        m = 1
        for c in range(NCORES):
            if j < len(core_slots[c]):
                m = max(m, groups_needed(core_slots[c][j][3]))
        L.append(m)
    A = np.concatenate([[0], np.cumsum(L)]).astype(np.int64)  # slot -> group start
    ngroups = int(A[-1])
    ntiles = (ngroups + 127) // 128
    ntok = ngroups * G

    # Per-core token-gather indices (into flat [B*S]) and membership weights.
    tok_idx = np.full((NCORES, ntok), -1, dtype=np.int64)
    member = np.zeros((NCORES, 128, ntiles, seg_cap), dtype=np.float32)
    out_map = np.full((NCORES, seg_cap), -1, dtype=np.int64)
    for c in range(NCORES):
        for j, (grow, b, s0, cnt) in enumerate(core_slots[c]):
            out_map[c, j] = grow
            g0 = int(A[j])
            nfull, rem = divmod(cnt, G)
            base = b * S + s0
            t0 = base  # first token, used as the harmless duplicate
            pos = g0 * G
            tok_idx[c, pos:pos + cnt] = np.arange(base, base + cnt)
            pos += cnt
            npure = L[j] - nfull - (1 if rem else 0)
            r = (G - rem) % G
            if r:
                tok_idx[c, pos:pos + r] = t0
                pos += r
            if npure:
                tok_idx[c, pos:pos + npure * G] = t0
            # weights: real groups 1/cnt, pure groups -r/(npure*G*cnt)
            inv = 1.0 / cnt
            nreal = nfull + (1 if rem else 0)
            for k in range(nreal):
                g = g0 + k
                member[c, g % 128, g // 128, j] = inv
            beta = -r / (npure * G) * inv if (npure and r) else 0.0
            for k in range(npure):
                g = g0 + nreal + k
                member[c, g % 128, g // 128, j] = beta
    return {
        "core_slots": core_slots,
        "seg_cap": seg_cap,
        "L": L,
        "A": A,
        "ntiles": ntiles,
        "ntok": ntok,
        "tok_idx": tok_idx,
        "member": member,
        "out_map": out_map,
        "nrows": Bn * Tn,
    }


def _build_program(ntiles, seg_cap, A, L):
    """Emit the SPMD Bass program (identical for all cores)."""
    ngroups = int(A[-1])
    ntok = ngroups * G

    nc = bacc.Bacc("TRN2", target_bir_lowering=False, debug=False,
                   num_devices=NCORES)
    hid = nc.dram_tensor("hid", [ntok, H], F16, kind="ExternalInput")
    mem = nc.dram_tensor("mem", [128, ntiles, seg_cap], F16, kind="ExternalInput")
    wt = nc.dram_tensor("wt", [2 * H, D_OUT], F16, kind="ExternalInput")
    bvec = nc.dram_tensor("bvec", [1, D_OUT], F16, kind="ExternalInput")
    out = nc.dram_tensor("out", [seg_cap, D_OUT], F32, kind="ExternalOutput")

    # slots' final max reduce is emitted right after the last tile covering
    # them; adjacent slots with equal group counts share one batched reduce
    # (their group ranges are contiguous since A is the cumsum of L).
    cover = [[] for _ in range(ntiles)]
    for j in range(seg_cap):
        cover[(int(A[j]) + int(L[j]) - 1) // 128].append(j)
    rbatches = [[] for _ in range(ntiles)]
    for t in range(ntiles):
        run = []
        for j in sorted(cover[t]):
            if run and j == run[-1] + 1 and int(L[j]) == int(L[run[0]]):
                run.append(j)
            else:
                if run:
                    rbatches[t].append(run)
                run = [j]
        if run:
            rbatches[t].append(run)

    with tile.TileContext(nc) as tc:
        with (
            tc.tile_pool(name="const", bufs=1) as constp,
            tc.tile_pool(name="hidp", bufs=3) as hidp,
            tc.tile_pool(name="partial", bufs=3) as partp,
            tc.tile_pool(name="tmp", bufs=2) as tmpp,
            tc.tile_pool(name="psum_tr", bufs=2, space="PSUM") as trpp,
            tc.tile_pool(name="psum_acc", bufs=1, space="PSUM") as accp,
            tc.tile_pool(name="small", bufs=1) as smallp,
        ):
            ident = constp.tile([128, 128], F16)
            make_identity(nc, ident[:])

            # W chunks are emitted on the SAME sync queue as the hid tiles,
            # but AFTER them (see post-loop) — the queue's FIFO order back-
            # loads W behind the token stream, and the kb-outer GEMM chases
            # the chunks as they land.
            wt_sb = constp.tile([128, KB, D_OUT], F16)
            wt_view = wt[:].rearrange("(kb p) n -> p kb n", p=128)
            b_sb = constp.tile([1, D_OUT], F16)
            nc.scalar.dma_start(out=b_sb[:], in_=bvec[:])
            ones_sb = constp.tile([1, seg_cap], F16)
            nc.gpsimd.memset(ones_sb[:], 1.0)
            # mem rides the scalar HWDGE queue so the sync queue's first
            # transfer is hid tile 0.
            mem_sb = constp.tile([128, ntiles, seg_cap], F16)
            nc.scalar.dma_start(out=mem_sb[:], in_=mem[:])

            trmax = constp.tile([128, HB, ngroups], F16)
            # featT[:, kb, j]: kb 0..HB-1 = max features, HB..2HB-1 = means
            featT = smallp.tile([128, KB, seg_cap], F16)
            mean_ps = accp.tile([seg_cap, D_OUT], F32, tag="acc")

            def emit_reduces(t):
                for run in rbatches[t]:
                    j0, m, l = run[0], len(run), int(L[run[0]])
                    a0 = int(A[j0])
                    if m == 1:
                        nc.vector.reduce_max(
                            out=featT[:, :HB, j0],
                            in_=trmax[:, :, a0:a0 + l],
                            axis=mybir.AxisListType.X,
                        )
                    else:
                        nc.vector.reduce_max(
                            out=featT[:, :HB, j0:j0 + m],
                            in_=trmax[:, :, a0:a0 + m * l]
                                .rearrange("p b (m l) -> p b m l", l=l),
                            axis=mybir.AxisListType.X,
                        )

            # Software pipeline: each engine's per-tile work-list only touches
            # data produced >= one stage earlier, so queues never stall on
            # same-tile cross-engine chains.
            #   iter t: [dma t] [DVE tree t] [PE means t] [PE transposes t-1]
            #           [Act copy t-1] [DVE reduces t-2]
            gmaxs = {}

            def stage_front(t):
                pt = min(128, ngroups - t * 128)  # groups in this tile
                ht = hidp.tile([128, G * H], F16)
                nc.sync.dma_start(
                    out=ht[:pt, :],
                    in_=hid[t * PTILE:t * PTILE + pt * G, :]
                        .rearrange("(p g) h -> p (g h)", g=G),
                )
                # Max tree over G=8 tokens/partition on DVE; the whole sum
                # tree is folded into the PE accumulation (8 raw blocks).
                gmax = partp.tile([128, H], F16, tag="gmax")
                mx1 = tmpp.tile([128, 4 * H], F16, tag="mx1")
                mx2 = tmpp.tile([128, 2 * H], F16, tag="mx2")
                half = G // 2 * H
                nc.vector.tensor_tensor(out=mx1[:pt], in0=ht[:pt, :half],
                                        in1=ht[:pt, half:], op=mybir.AluOpType.max)
                nc.vector.tensor_tensor(out=mx2[:pt], in0=mx1[:pt, :2 * H],
                                        in1=mx1[:pt, 2 * H:],
                                        op=mybir.AluOpType.max)
                nc.vector.tensor_tensor(out=gmax[:pt], in0=mx2[:pt, :H],
                                        in1=mx2[:pt, H:2 * H],
                                        op=mybir.AluOpType.max)
                gmaxs[t] = (gmax, pt)
                # segment means accumulate on PE (weights already carry 1/cnt)
                for nh in range(2):
                    for hv in range(G):
                        rhs = ht[:pt, hv * H + nh * 512:hv * H + nh * 512 + 512]
                        nc.tensor.matmul(
                            mean_ps[:, nh * 512:(nh + 1) * 512],
                            lhsT=mem_sb[:pt, t, :],
                            rhs=rhs,
                            start=(t == 0 and hv == 0),
                            stop=(t == ntiles - 1 and hv == G - 1),
                        )

            def stage_transpose(t):
                gmax, pt = gmaxs.pop(t)
                trp = trpp.tile([128, H], F16, tag="trp")
                for hb in range(HB):
                    nc.tensor.transpose(
                        trp[:, hb * 128:hb * 128 + pt],
                        gmax[:pt, hb * 128:(hb + 1) * 128],
                        ident[:pt, :pt],
                    )
                nc.scalar.copy(
                    out=trmax[:, :, t * 128:t * 128 + pt],
                    in_=trp[:].rearrange("p (b g) -> p b g", g=128)[:, :, :pt],
                )

            for t in range(ntiles):
                stage_front(t)
                if t >= 1:
                    stage_transpose(t - 1)
                if t >= 2:
                    emit_reduces(t - 2)
            # W streams in behind the hid tiles on the same queue (FIFO);
            # the kb-outer GEMM chases the chunks as they land.
            for wch in range(HB):
                nc.sync.dma_start(
                    out=wt_sb[:, 2 * wch:2 * wch + 2, :],
                    in_=wt_view[:, 2 * wch:2 * wch + 2, :],
                )
            stage_transpose(ntiles - 1)
            if ntiles >= 2:
                emit_reduces(ntiles - 2)
            emit_reduces(ntiles - 1)

            # means: PSUM -> SBUF fp16, transpose into featT[:, HB:, :]
            means = smallp.tile([seg_cap, D_OUT], F16)
            nc.scalar.copy(out=means[:], in_=mean_ps[:])
            tr2 = trpp.tile([128, HB * seg_cap], F16, tag="tr2")
            for hb in range(HB):
                nc.tensor.transpose(
                    tr2[:, hb * seg_cap:(hb + 1) * seg_cap],
                    means[:, hb * 128:(hb + 1) * 128],
                    ident[:seg_cap, :seg_cap],
                )
            nc.scalar.copy(
                out=featT[:, HB:, :],
                in_=tr2[:].rearrange("p (b j) -> p b j", j=seg_cap),
            )

            # GEMM: out[slot, n] = tanh(sum_k featT[k, slot] * wt[k, n] + b[n])
            # bias enters as a rank-1 start matmul (ones^T @ b broadcast).
            # kb-outer order so each k-block is consumed as its W chunk lands.
            osb = smallp.tile([seg_cap, D_OUT], F32)
            gem_ps = trpp.tile([seg_cap, D_OUT], F32, tag="trp")
            for nh in range(2):
                nsl = slice(nh * 512, (nh + 1) * 512)
                nc.tensor.matmul(gem_ps[:, nsl], lhsT=ones_sb[:, :],
                                 rhs=b_sb[:, nsl], start=True, stop=False)
            for kb in range(KB):
                for nh in range(2):
                    nsl = slice(nh * 512, (nh + 1) * 512)
                    nc.tensor.matmul(
                        gem_ps[:, nsl],
                        lhsT=featT[:, kb, :],
                        rhs=wt_sb[:, kb, nsl],
                        start=False,
                        stop=(kb == KB - 1),
                    )
            nc.scalar.activation(osb[:], gem_ps[:],
                                 mybir.ActivationFunctionType.Tanh)
            nc.sync.dma_start(out=out[:], in_=osb[:])

    nc.compile()
    return nc


def _build_in_maps(sched, hidden_states, W, b):
    seg_cap, ntiles = sched["seg_cap"], sched["ntiles"]
    flat = np.asarray(hidden_states, dtype=np.float32).reshape(B * S, H)
    flat16 = flat.astype(np.float16)
    wt_np = np.ascontiguousarray(
        np.asarray(W, dtype=np.float32).T.astype(np.float16))  # [2H, D]
    b_np = np.asarray(b, dtype=np.float32).astype(np.float16).reshape(1, D_OUT)

    in_maps = []
    for c in range(NCORES):
        idx = sched["tok_idx"][c]
        stream = np.zeros((sched["ntok"], H), dtype=np.float16)
        valid = idx >= 0
        stream[valid] = flat16[idx[valid]]
        memc = np.ascontiguousarray(
            sched["member"][c].reshape(128, ntiles, seg_cap).astype(np.float16))
        in_maps.append({
            "hid": stream,
            "mem": memc,
            "wt": wt_np,
            "bvec": b_np,
        })
    return in_maps


def kernel(hidden_states, W, b, turns, parts):
    parts = np.asarray(parts)
    turns = np.asarray(turns)

    sched = _build_schedule(parts, turns)
    nc = _build_program(sched["ntiles"], sched["seg_cap"],
                        sched["A"], sched["L"])
    in_maps = _build_in_maps(sched, hidden_states, W, b)

    res = run_bass_kernel_spmd(nc, in_maps, list(range(NCORES)))

    full = np.zeros((sched["nrows"], D_OUT), dtype=np.float32)
    for c in range(NCORES):
        oc = res.results[c]["out"]
        for j in range(sched["seg_cap"]):
            g = sched["out_map"][c, j]
            if g >= 0:
                full[g] = oc[j]
    return full


# revision 24
# speedup vs baseline: 1.0572x; 1.0572x over previous
"""Trainium2 Bass kernel for nn_BartPooler_53815940219079 (segment_reduce).

Computes, for each of B*T segments of a [B, S, H] hidden-state tensor:
  feat = concat([segment_max, segment_mean])  -> tanh(feat @ W.T + b)

Strategy (8 NeuronCores, SPMD — one program, per-core data):
  * Host compacts each segment's used tokens into a per-core fp16 token
    stream, padding every segment with duplicates of its first token so
    each segment occupies a whole number of G-token "groups" (plus a
    compensation group whose negative membership weight cancels the
    duplicate tokens in the sum).  Segments are dealt snake-wise across
    cores by size so all cores share one static layout.
  * Device, per 128-group tile: grouped max/sum partials (DVE fp16);
    per-segment mean accumulation via membership matmuls on PE (weights
    fold in 1/count, fp32 PSUM); PE transposes of the max partials; per-
    segment max reduce on DVE; then a fused [2H] x [2H, D] fp16 GEMM
    with bias folded in as a rank-1 matmul, tanh on ScalarE.
"""

import numpy as np

import concourse.bacc as bacc
import concourse.mybir as mybir
import concourse.tile as tile
from concourse.bass_utils import run_bass_kernel_spmd
from concourse.masks import make_identity

NCORES = 8
G = 4          # tokens per group
PTILE = 128 * G  # tokens per main tile

B, S, H, T = 16, 4096, 1024, 16
D_OUT = 1024
HB = H // 128  # h-blocks per hidden vector
KB = 2 * HB    # k-blocks in the feature GEMM

F32 = mybir.dt.float32
F16 = mybir.dt.float16


def _build_schedule(parts, turns):
    """Host-side: segment list -> per-core compacted layout (uniform shapes)."""
    Bn, Tn = parts.shape
    segs = []  # (global_row, example, start_token, count)
    for b in range(Bn):
        cum = 0
        for j in range(Tn):
            c = int(parts[b, j])
            if j < int(turns[b]):
                segs.append((b * Tn + j, b, 1 + cum, c))
            cum += c

    # Deal segments to cores by size rank: slot j holds the 8 segments of
    # ranks [8j, 8j+8), one per core, so the uniform per-slot group count
    # L[j] (max over cores) is as tight as possible.
    order = sorted(range(len(segs)), key=lambda i: -segs[i][3])
    core_slots = [[] for _ in range(NCORES)]
    for rank, i in enumerate(order):
        core_slots[rank % NCORES].append(segs[i])
    seg_cap = max(len(s) for s in core_slots)

    def groups_needed(cnt):
        g = (cnt + G - 1) // G
        if cnt % G:
            g += 1  # at least one pure-duplicate group for the compensation
        return g

    # Uniform per-slot group counts across cores.
    L = []
    for j in range(seg_cap):
        m = 1
        for c in range(NCORES):
            if j < len(core_slots[c]):
                m = max(m, groups_needed(core_slots[c][j][3]))
        L.append(m)
    A = np.concatenate([[0], np.cumsum(L)]).astype(np.int64)  # slot -> group start
    ngroups = int(A[-1])
    ntiles = (ngroups + 127) // 128
    ntok = ngroups * G

    # Per-core token-gather indices (into flat [B*S]) and membership weights.
    tok_idx = np.full((NCORES, ntok), -1, dtype=np.int64)
    member = np.zeros((NCORES, 128, ntiles, seg_cap), dtype=np.float32)
    out_map = np.full((NCORES, seg_cap), -1, dtype=np.int64)
    for c in range(NCORES):
        for j, (grow, b, s0, cnt) in enumerate(core_slots[c]):
            out_map[c, j] = grow
            g0 = int(A[j])
            nfull, rem = divmod(cnt, G)
            base = b * S + s0
            t0 = base  # first token, used as the harmless duplicate
            pos = g0 * G
            tok_idx[c, pos:pos + cnt] = np.arange(base, base + cnt)
            pos += cnt
            npure = L[j] - nfull - (1 if rem else 0)
            r = (G - rem) % G
            if r:
                tok_idx[c, pos:pos + r] = t0
                pos += r
            if npure:
                tok_idx[c, pos:pos + npure * G] = t0
            # weights: real groups 1/cnt, pure groups -r/(npure*G*cnt)
            inv = 1.0 / cnt
            nreal = nfull + (1 if rem else 0)
            for k in range(nreal):
                g = g0 + k
                member[c, g % 128, g // 128, j] = inv
            beta = -r / (npure * G) * inv if (npure and r) else 0.0
            for k in range(npure):
                g = g0 + nreal + k
                member[c, g % 128, g // 128, j] = beta
    return {
        "core_slots": core_slots,
        "seg_cap": seg_cap,
        "L": L,
        "A": A,
        "ntiles": ntiles,
        "ntok": ntok,
        "tok_idx": tok_idx,
        "member": member,
        "out_map": out_map,
        "nrows": Bn * Tn,
    }


def _build_program(ntiles, seg_cap, A, L):
    """Emit the SPMD Bass program (identical for all cores)."""
    ngroups = int(A[-1])
    ntok = ngroups * G

    nc = bacc.Bacc("TRN2", target_bir_lowering=False, debug=False,
                   num_devices=NCORES)
    hid = nc.dram_tensor("hid", [ntok, H], F16, kind="ExternalInput")
    mem = nc.dram_tensor("mem", [128, ntiles, seg_cap], F16, kind="ExternalInput")
    wt = nc.dram_tensor("wt", [2 * H, D_OUT], F16, kind="ExternalInput")
    bvec = nc.dram_tensor("bvec", [1, D_OUT], F16, kind="ExternalInput")
    out = nc.dram_tensor("out", [seg_cap, D_OUT], F32, kind="ExternalOutput")

    # slots' final max reduce is emitted right after the last tile covering
    # them; adjacent slots with equal group counts share one batched reduce
    # (their group ranges are contiguous since A is the cumsum of L).
    cover = [[] for _ in range(ntiles)]
    for j in range(seg_cap):
        cover[(int(A[j]) + int(L[j]) - 1) // 128].append(j)
    rbatches = [[] for _ in range(ntiles)]
    for t in range(ntiles):
        run = []
        for j in sorted(cover[t]):
            if run and j == run[-1] + 1 and int(L[j]) == int(L[run[0]]):
                run.append(j)
            else:
                if run:
                    rbatches[t].append(run)
                run = [j]
        if run:
            rbatches[t].append(run)

    with tile.TileContext(nc) as tc:
        with (
            tc.tile_pool(name="const", bufs=1) as constp,
            tc.tile_pool(name="hidp", bufs=3) as hidp,
            tc.tile_pool(name="partial", bufs=3) as partp,
            tc.tile_pool(name="psum_tr", bufs=2, space="PSUM") as trpp,
            tc.tile_pool(name="psum_acc", bufs=1, space="PSUM") as accp,
            tc.tile_pool(name="small", bufs=1) as smallp,
        ):
            ident = constp.tile([128, 128], F16)
            make_identity(nc, ident[:])

            # W chunks are emitted on the SAME sync queue as the hid tiles,
            # but AFTER them (see post-loop) — the queue's FIFO order back-
            # loads W behind the token stream, and the kb-outer GEMM chases
            # the chunks as they land.
            wt_sb = constp.tile([128, KB, D_OUT], F16)
            wt_view = wt[:].rearrange("(kb p) n -> p kb n", p=128)
            b_sb = constp.tile([1, D_OUT], F16)
            nc.scalar.dma_start(out=b_sb[:], in_=bvec[:])
            ones_sb = constp.tile([1, seg_cap], F16)
            nc.gpsimd.memset(ones_sb[:], 1.0)
            # mem rides the scalar HWDGE queue so the sync queue's first
            # transfer is hid tile 0.
            mem_sb = constp.tile([128, ntiles, seg_cap], F16)
            nc.scalar.dma_start(out=mem_sb[:], in_=mem[:])

            trmax = constp.tile([128, HB, ngroups], F16)
            # featT[:, kb, j]: kb 0..HB-1 = max features, HB..2HB-1 = means
            featT = smallp.tile([128, KB, seg_cap], F16)
            mean_ps = accp.tile([seg_cap, D_OUT], F32, tag="acc")

            def emit_reduces(t):
                for run in rbatches[t]:
                    j0, m, l = run[0], len(run), int(L[run[0]])
                    a0 = int(A[j0])
                    if m == 1:
                        nc.vector.reduce_max(
                            out=featT[:, :HB, j0],
                            in_=trmax[:, :, a0:a0 + l],
                            axis=mybir.AxisListType.X,
                        )
                    else:
                        nc.vector.reduce_max(
                            out=featT[:, :HB, j0:j0 + m],
                            in_=trmax[:, :, a0:a0 + m * l]
                                .rearrange("p b (m l) -> p b m l", l=l),
                            axis=mybir.AxisListType.X,
                        )

            # Software pipeline: each engine's per-tile work-list only touches
            # data produced >= one stage earlier, so queues never stall on
            # same-tile cross-engine chains.
            #   iter t: [dma t] [DVE tree t] [PE means t] [PE transposes t-1]
            #           [Act copy t-1] [DVE reduces t-2]
            gmaxs = {}

            def stage_front(t):
                pt = min(128, ngroups - t * 128)  # groups in this tile
                ht = hidp.tile([128, G * H], F16)
                nc.sync.dma_start(
                    out=ht[:pt, :],
                    in_=hid[t * PTILE:t * PTILE + pt * G, :]
                        .rearrange("(p g) h -> p (g h)", g=G),
                )
                # Max tree over G=4 tokens/partition on DVE.  For the sum,
                # only blocks 0+2 are pre-added on DVE (tsm); blocks 1 and 3
                # feed PE raw — the PSUM accumulation absorbs the rest of the
                # sum tree, balancing DVE vs PE against the DMA pace.
                gmax = partp.tile([128, H], F16, tag="gmax")
                mx1 = partp.tile([128, 2 * H], F16, tag="mx1")
                tsm = partp.tile([128, H], F16, tag="tsm")
                half = G // 2 * H
                nc.vector.tensor_tensor(out=tsm[:pt], in0=ht[:pt, :H],
                                        in1=ht[:pt, 2 * H:3 * H],
                                        op=mybir.AluOpType.add)
                nc.vector.tensor_tensor(out=mx1[:pt], in0=ht[:pt, :half],
                                        in1=ht[:pt, half:], op=mybir.AluOpType.max)
                nc.vector.tensor_tensor(out=gmax[:pt], in0=mx1[:pt, :H],
                                        in1=mx1[:pt, H:2 * H],
                                        op=mybir.AluOpType.max)
                gmaxs[t] = (gmax, pt)
                # segment means accumulate on PE (weights already carry 1/cnt)
                for nh in range(2):
                    rhss = (tsm[:pt, nh * 512:nh * 512 + 512],
                            ht[:pt, H + nh * 512:H + nh * 512 + 512],
                            ht[:pt, 3 * H + nh * 512:3 * H + nh * 512 + 512])
                    for hv, rhs in enumerate(rhss):
                        nc.tensor.matmul(
                            mean_ps[:, nh * 512:(nh + 1) * 512],
                            lhsT=mem_sb[:pt, t, :],
                            rhs=rhs,
                            start=(t == 0 and hv == 0),
                            stop=(t == ntiles - 1 and hv == 2),
                        )

            def stage_transpose(t):
                gmax, pt = gmaxs.pop(t)
                trp = trpp.tile([128, H], F16, tag="trp")
                for hb in range(HB):
                    nc.tensor.transpose(
                        trp[:, hb * 128:hb * 128 + pt],
                        gmax[:pt, hb * 128:(hb + 1) * 128],
                        ident[:pt, :pt],
                    )
                nc.scalar.copy(
                    out=trmax[:, :, t * 128:t * 128 + pt],
                    in_=trp[:].rearrange("p (b g) -> p b g", g=128)[:, :, :pt],
                )

            for t in range(ntiles):
                stage_front(t)
                if t >= 1:
                    stage_transpose(t - 1)
                if t >= 2:
                    emit_reduces(t - 2)
            # W streams in behind the hid tiles on the same queue (FIFO);
            # the kb-outer GEMM chases the chunks as they land.
            for wch in range(HB):
                nc.sync.dma_start(
                    out=wt_sb[:, 2 * wch:2 * wch + 2, :],
                    in_=wt_view[:, 2 * wch:2 * wch + 2, :],
                )
            stage_transpose(ntiles - 1)
            if ntiles >= 2:
                emit_reduces(ntiles - 2)
            emit_reduces(ntiles - 1)

            # means: PSUM -> SBUF fp16, transpose into featT[:, HB:, :]
            means = smallp.tile([seg_cap, D_OUT], F16)
            nc.scalar.copy(out=means[:], in_=mean_ps[:])
            tr2 = trpp.tile([128, HB * seg_cap], F16, tag="tr2")
            for hb in range(HB):
                nc.tensor.transpose(
                    tr2[:, hb * seg_cap:(hb + 1) * seg_cap],
                    means[:, hb * 128:(hb + 1) * 128],
                    ident[:seg_cap, :seg_cap],
                )
            nc.scalar.copy(
                out=featT[:, HB:, :],
                in_=tr2[:].rearrange("p (b j) -> p b j", j=seg_cap),
            )

            # GEMM: out[slot, n] = tanh(sum_k featT[k, slot] * wt[k, n] + b[n])
            # bias enters as a rank-1 start matmul (ones^T @ b broadcast).
            # kb-outer order so each k-block is consumed as its W chunk lands.
            osb = smallp.tile([seg_cap, D_OUT], F32)
            gem_ps = trpp.tile([seg_cap, D_OUT], F32, tag="trp")
            for nh in range(2):
                nsl = slice(nh * 512, (nh + 1) * 512)
                nc.tensor.matmul(gem_ps[:, nsl], lhsT=ones_sb[:, :],
                                 rhs=b_sb[:, nsl], start=True, stop=False)
            for kb in range(KB):
                for nh in range(2):
                    nsl = slice(nh * 512, (nh + 1) * 512)
                    nc.tensor.matmul(
                        gem_ps[:, nsl],
                        lhsT=featT[:, kb, :],
                        rhs=wt_sb[:, kb, nsl],
                        start=False,
                        stop=(kb == KB - 1),
                    )
            nc.scalar.activation(osb[:], gem_ps[:],
                                 mybir.ActivationFunctionType.Tanh)
            nc.sync.dma_start(out=out[:], in_=osb[:])

    nc.compile()
    return nc


def _build_in_maps(sched, hidden_states, W, b):
    seg_cap, ntiles = sched["seg_cap"], sched["ntiles"]
    flat = np.asarray(hidden_states, dtype=np.float32).reshape(B * S, H)
    flat16 = flat.astype(np.float16)
    wt_np = np.ascontiguousarray(
        np.asarray(W, dtype=np.float32).T.astype(np.float16))  # [2H, D]
    b_np = np.asarray(b, dtype=np.float32).astype(np.float16).reshape(1, D_OUT)

    in_maps = []
    for c in range(NCORES):
        idx = sched["tok_idx"][c]
        stream = np.zeros((sched["ntok"], H), dtype=np.float16)
        valid = idx >= 0
        stream[valid] = flat16[idx[valid]]
        memc = np.ascontiguousarray(
            sched["member"][c].reshape(128, ntiles, seg_cap).astype(np.float16))
        in_maps.append({
            "hid": stream,
            "mem": memc,
            "wt": wt_np,
            "bvec": b_np,
        })
    return in_maps


def kernel(hidden_states, W, b, turns, parts):
    parts = np.asarray(parts)
    turns = np.asarray(turns)

    sched = _build_schedule(parts, turns)
    nc = _build_program(sched["ntiles"], sched["seg_cap"],
                        sched["A"], sched["L"])
    in_maps = _build_in_maps(sched, hidden_states, W, b)

    res = run_bass_kernel_spmd(nc, in_maps, list(range(NCORES)))

    full = np.zeros((sched["nrows"], D_OUT), dtype=np.float32)
    for c in range(NCORES):
        oc = res.results[c]["out"]
        for j in range(sched["seg_cap"]):
            g = sched["out_map"][c, j]
            if g >= 0:
                full[g] = oc[j]
    return full


# revision 25
# speedup vs baseline: 1.0776x; 1.0194x over previous
"""Trainium2 Bass kernel for nn_BartPooler_53815940219079 (segment_reduce).

Computes, for each of B*T segments of a [B, S, H] hidden-state tensor:
  feat = concat([segment_max, segment_mean])  -> tanh(feat @ W.T + b)

Strategy (8 NeuronCores, SPMD — one program, per-core data):
  * Host compacts each segment's used tokens into a per-core fp16 token
    stream, padding every segment with duplicates of its first token so
    each segment occupies a whole number of G-token "groups" (plus a
    compensation group whose negative membership weight cancels the
    duplicate tokens in the sum).  Segments are dealt snake-wise across
    cores by size so all cores share one static layout.
  * Device, per 128-group tile: grouped max/sum partials (DVE fp16);
    per-segment mean accumulation via membership matmuls on PE (weights
    fold in 1/count, fp32 PSUM); PE transposes of the max partials; per-
    segment max reduce on DVE; then a fused [2H] x [2H, D] fp16 GEMM
    with bias folded in as a rank-1 matmul, tanh on ScalarE.
"""

import numpy as np

import concourse.bacc as bacc
import concourse.mybir as mybir
import concourse.tile as tile
from concourse.bass_utils import run_bass_kernel_spmd
from concourse.masks import make_identity

NCORES = 8
G = 4          # tokens per group
PTILE = 128 * G  # tokens per main tile

B, S, H, T = 16, 4096, 1024, 16
D_OUT = 1024
HB = H // 128  # h-blocks per hidden vector
KB = 2 * HB    # k-blocks in the feature GEMM

F32 = mybir.dt.float32
F16 = mybir.dt.float16


def _build_schedule(parts, turns):
    """Host-side: segment list -> per-core compacted layout (uniform shapes)."""
    Bn, Tn = parts.shape
    segs = []  # (global_row, example, start_token, count)
    for b in range(Bn):
        cum = 0
        for j in range(Tn):
            c = int(parts[b, j])
            if j < int(turns[b]):
                segs.append((b * Tn + j, b, 1 + cum, c))
            cum += c

    # Deal segments to cores by size rank: slot j holds the 8 segments of
    # ranks [8j, 8j+8), one per core, so the uniform per-slot group count
    # L[j] (max over cores) is as tight as possible.
    order = sorted(range(len(segs)), key=lambda i: -segs[i][3])
    core_slots = [[] for _ in range(NCORES)]
    for rank, i in enumerate(order):
        core_slots[rank % NCORES].append(segs[i])
    seg_cap = max(len(s) for s in core_slots)

    def groups_needed(cnt):
        g = (cnt + G - 1) // G
        if cnt % G:
            g += 1  # at least one pure-duplicate group for the compensation
        return g

    # Uniform per-slot group counts across cores.
    L = []
    for j in range(seg_cap):
        m = 1
        for c in range(NCORES):
            if j < len(core_slots[c]):
                m = max(m, groups_needed(core_slots[c][j][3]))
        L.append(m)
    A = np.concatenate([[0], np.cumsum(L)]).astype(np.int64)  # slot -> group start
    ngroups = int(A[-1])
    ntiles = (ngroups + 127) // 128
    ntok = ngroups * G

    # Per-core token-gather indices (into flat [B*S]) and membership weights.
    tok_idx = np.full((NCORES, ntok), -1, dtype=np.int64)
    member = np.zeros((NCORES, 128, ntiles, seg_cap), dtype=np.float32)
    out_map = np.full((NCORES, seg_cap), -1, dtype=np.int64)
    for c in range(NCORES):
        for j, (grow, b, s0, cnt) in enumerate(core_slots[c]):
            out_map[c, j] = grow
            g0 = int(A[j])
            nfull, rem = divmod(cnt, G)
            base = b * S + s0
            t0 = base  # first token, used as the harmless duplicate
            pos = g0 * G
            tok_idx[c, pos:pos + cnt] = np.arange(base, base + cnt)
            pos += cnt
            npure = L[j] - nfull - (1 if rem else 0)
            r = (G - rem) % G
            if r:
                tok_idx[c, pos:pos + r] = t0
                pos += r
            if npure:
                tok_idx[c, pos:pos + npure * G] = t0
            # weights: real groups 1/cnt, pure groups -r/(npure*G*cnt)
            inv = 1.0 / cnt
            nreal = nfull + (1 if rem else 0)
            for k in range(nreal):
                g = g0 + k
                member[c, g % 128, g // 128, j] = inv
            beta = -r / (npure * G) * inv if (npure and r) else 0.0
            for k in range(npure):
                g = g0 + nreal + k
                member[c, g % 128, g // 128, j] = beta
    return {
        "core_slots": core_slots,
        "seg_cap": seg_cap,
        "L": L,
        "A": A,
        "ntiles": ntiles,
        "ntok": ntok,
        "tok_idx": tok_idx,
        "member": member,
        "out_map": out_map,
        "nrows": Bn * Tn,
    }


def _build_program(ntiles, seg_cap, A, L):
    """Emit the SPMD Bass program (identical for all cores)."""
    ngroups = int(A[-1])
    ntok = ngroups * G

    nc = bacc.Bacc("TRN2", target_bir_lowering=False, debug=False,
                   num_devices=NCORES)
    hid = nc.dram_tensor("hid", [ntok, H], F16, kind="ExternalInput")
    mem = nc.dram_tensor("mem", [128, ntiles, seg_cap], F16, kind="ExternalInput")
    wt = nc.dram_tensor("wt", [2 * H, D_OUT], F16, kind="ExternalInput")
    bvec = nc.dram_tensor("bvec", [1, D_OUT], F16, kind="ExternalInput")
    out = nc.dram_tensor("out", [seg_cap, D_OUT], F32, kind="ExternalOutput")

    # slots' final max reduce is emitted right after the last tile covering
    # them; adjacent slots with equal group counts share one batched reduce
    # (their group ranges are contiguous since A is the cumsum of L).
    cover = [[] for _ in range(ntiles)]
    for j in range(seg_cap):
        cover[(int(A[j]) + int(L[j]) - 1) // 128].append(j)
    rbatches = [[] for _ in range(ntiles)]
    for t in range(ntiles):
        run = []
        for j in sorted(cover[t]):
            if run and j == run[-1] + 1 and int(L[j]) == int(L[run[0]]):
                run.append(j)
            else:
                if run:
                    rbatches[t].append(run)
                run = [j]
        if run:
            rbatches[t].append(run)

    with tile.TileContext(nc) as tc:
        with (
            tc.tile_pool(name="const", bufs=1) as constp,
            tc.tile_pool(name="hidp", bufs=3) as hidp,
            tc.tile_pool(name="partial", bufs=3) as partp,
            tc.tile_pool(name="psum_tr", bufs=2, space="PSUM") as trpp,
            tc.tile_pool(name="psum_acc", bufs=1, space="PSUM") as accp,
            tc.tile_pool(name="small", bufs=1) as smallp,
        ):
            ident = constp.tile([128, 128], F16)
            make_identity(nc, ident[:])

            # W chunks are emitted on the SAME sync queue as the hid tiles,
            # but AFTER them (see post-loop) — the queue's FIFO order back-
            # loads W behind the token stream, and the kb-outer GEMM chases
            # the chunks as they land.
            wt_sb = constp.tile([128, KB, D_OUT], F16)
            wt_view = wt[:].rearrange("(kb p) n -> p kb n", p=128)
            b_sb = constp.tile([1, D_OUT], F16)
            nc.scalar.dma_start(out=b_sb[:], in_=bvec[:])
            ones_sb = constp.tile([1, seg_cap], F16)
            nc.gpsimd.memset(ones_sb[:], 1.0)
            # mem rides the scalar HWDGE queue so the sync queue's first
            # transfer is hid tile 0.
            mem_sb = constp.tile([128, ntiles, seg_cap], F16)
            nc.scalar.dma_start(out=mem_sb[:], in_=mem[:])

            trmax = constp.tile([128, HB, ngroups], F16)
            # featT[:, kb, j]: kb 0..HB-1 = max features, HB..2HB-1 = means
            featT = smallp.tile([128, KB, seg_cap], F16)
            mean_ps = accp.tile([seg_cap, D_OUT], F32, tag="acc")

            def emit_reduces(t):
                for run in rbatches[t]:
                    j0, m, l = run[0], len(run), int(L[run[0]])
                    a0 = int(A[j0])
                    if m == 1:
                        nc.vector.reduce_max(
                            out=featT[:, :HB, j0],
                            in_=trmax[:, :, a0:a0 + l],
                            axis=mybir.AxisListType.X,
                        )
                    else:
                        nc.vector.reduce_max(
                            out=featT[:, :HB, j0:j0 + m],
                            in_=trmax[:, :, a0:a0 + m * l]
                                .rearrange("p b (m l) -> p b m l", l=l),
                            axis=mybir.AxisListType.X,
                        )

            # Software pipeline: each engine's per-tile work-list only touches
            # data produced >= one stage earlier, so queues never stall on
            # same-tile cross-engine chains.
            #   iter t: [dma t] [DVE tree t] [PE means t] [PE transposes t-1]
            #           [Act copy t-1] [DVE reduces t-2]
            gmaxs = {}

            def stage_front(t):
                pt = min(128, ngroups - t * 128)  # groups in this tile
                ht = hidp.tile([128, G * H], F16)
                nc.sync.dma_start(
                    out=ht[:pt, :],
                    in_=hid[t * PTILE:t * PTILE + pt * G, :]
                        .rearrange("(p g) h -> p (g h)", g=G),
                )
                # Max tree over G=4 tokens/partition on DVE.  For the sum,
                # only blocks 0+2 are pre-added on DVE (tsm); blocks 1 and 3
                # feed PE raw — the PSUM accumulation absorbs the rest of the
                # sum tree, balancing DVE vs PE against the DMA pace.
                gmax = partp.tile([128, H], F16, tag="gmax")
                mx1 = partp.tile([128, 2 * H], F16, tag="mx1")
                tsm = partp.tile([128, H], F16, tag="tsm")
                half = G // 2 * H
                nc.vector.tensor_tensor(out=tsm[:pt], in0=ht[:pt, :H],
                                        in1=ht[:pt, 2 * H:3 * H],
                                        op=mybir.AluOpType.add)
                nc.vector.tensor_tensor(out=mx1[:pt], in0=ht[:pt, :half],
                                        in1=ht[:pt, half:], op=mybir.AluOpType.max)
                nc.vector.tensor_tensor(out=gmax[:pt], in0=mx1[:pt, :H],
                                        in1=mx1[:pt, H:2 * H],
                                        op=mybir.AluOpType.max)
                gmaxs[t] = (gmax, pt)
                return pt, ht, tsm

            def stage_mm(t, pt, ht, tsm):
                # segment means accumulate on PE (weights already carry
                # 1/cnt); the raw blocks go first — they only depend on the
                # DMA, so PE starts before DVE's tsm is ready.
                for nh in range(2):
                    rhss = (ht[:pt, H + nh * 512:H + nh * 512 + 512],
                            ht[:pt, 3 * H + nh * 512:3 * H + nh * 512 + 512],
                            tsm[:pt, nh * 512:nh * 512 + 512])
                    for hv, rhs in enumerate(rhss):
                        nc.tensor.matmul(
                            mean_ps[:, nh * 512:(nh + 1) * 512],
                            lhsT=mem_sb[:pt, t, :],
                            rhs=rhs,
                            start=(t == 0 and hv == 0),
                            stop=(t == ntiles - 1 and hv == 2),
                        )

            def stage_transpose(t):
                gmax, pt = gmaxs.pop(t)
                trp = trpp.tile([128, H], F16, tag="trp")
                for hb in range(HB):
                    nc.tensor.transpose(
                        trp[:, hb * 128:hb * 128 + pt],
                        gmax[:pt, hb * 128:(hb + 1) * 128],
                        ident[:pt, :pt],
                    )
                nc.scalar.copy(
                    out=trmax[:, :, t * 128:t * 128 + pt],
                    in_=trp[:].rearrange("p (b g) -> p b g", g=128)[:, :, :pt],
                )

            for t in range(ntiles):
                args = stage_front(t)
                if t >= 1:
                    stage_transpose(t - 1)
                stage_mm(t, *args)
                if t >= 2:
                    emit_reduces(t - 2)
            # W streams in behind the hid tiles on the same queue (FIFO);
            # the kb-outer GEMM chases the chunks as they land.
            for wch in range(HB):
                nc.sync.dma_start(
                    out=wt_sb[:, 2 * wch:2 * wch + 2, :],
                    in_=wt_view[:, 2 * wch:2 * wch + 2, :],
                )
            stage_transpose(ntiles - 1)
            if ntiles >= 2:
                emit_reduces(ntiles - 2)
            emit_reduces(ntiles - 1)

            # means: PSUM -> SBUF fp16, transpose into featT[:, HB:, :]
            means = smallp.tile([seg_cap, D_OUT], F16)
            nc.scalar.copy(out=means[:], in_=mean_ps[:])
            tr2 = trpp.tile([128, HB * seg_cap], F16, tag="tr2")
            for hb in range(HB):
                nc.tensor.transpose(
                    tr2[:, hb * seg_cap:(hb + 1) * seg_cap],
                    means[:, hb * 128:(hb + 1) * 128],
                    ident[:seg_cap, :seg_cap],
                )
            nc.scalar.copy(
                out=featT[:, HB:, :],
                in_=tr2[:].rearrange("p (b j) -> p b j", j=seg_cap),
            )

            # GEMM: out[slot, n] = tanh(sum_k featT[k, slot] * wt[k, n] + b[n])
            # bias enters as a rank-1 start matmul (ones^T @ b broadcast).
            # kb-outer order so each k-block is consumed as its W chunk lands.
            osb = smallp.tile([seg_cap, D_OUT], F32)
            gem_ps = trpp.tile([seg_cap, D_OUT], F32, tag="trp")
            for nh in range(2):
                nsl = slice(nh * 512, (nh + 1) * 512)
                nc.tensor.matmul(gem_ps[:, nsl], lhsT=ones_sb[:, :],
                                 rhs=b_sb[:, nsl], start=True, stop=False)
            for kb in range(KB):
                for nh in range(2):
                    nsl = slice(nh * 512, (nh + 1) * 512)
                    nc.tensor.matmul(
                        gem_ps[:, nsl],
                        lhsT=featT[:, kb, :],
                        rhs=wt_sb[:, kb, nsl],
                        start=False,
                        stop=(kb == KB - 1),
                    )
            nc.scalar.activation(osb[:], gem_ps[:],
                                 mybir.ActivationFunctionType.Tanh)
            nc.sync.dma_start(out=out[:], in_=osb[:])

    nc.compile()
    return nc


def _build_in_maps(sched, hidden_states, W, b):
    seg_cap, ntiles = sched["seg_cap"], sched["ntiles"]
    flat = np.asarray(hidden_states, dtype=np.float32).reshape(B * S, H)
    flat16 = flat.astype(np.float16)
    wt_np = np.ascontiguousarray(
        np.asarray(W, dtype=np.float32).T.astype(np.float16))  # [2H, D]
    b_np = np.asarray(b, dtype=np.float32).astype(np.float16).reshape(1, D_OUT)

    in_maps = []
    for c in range(NCORES):
        idx = sched["tok_idx"][c]
        stream = np.zeros((sched["ntok"], H), dtype=np.float16)
        valid = idx >= 0
        stream[valid] = flat16[idx[valid]]
        memc = np.ascontiguousarray(
            sched["member"][c].reshape(128, ntiles, seg_cap).astype(np.float16))
        in_maps.append({
            "hid": stream,
            "mem": memc,
            "wt": wt_np,
            "bvec": b_np,
        })
    return in_maps


def kernel(hidden_states, W, b, turns, parts):
    parts = np.asarray(parts)
    turns = np.asarray(turns)

    sched = _build_schedule(parts, turns)
    nc = _build_program(sched["ntiles"], sched["seg_cap"],
                        sched["A"], sched["L"])
    in_maps = _build_in_maps(sched, hidden_states, W, b)

    res = run_bass_kernel_spmd(nc, in_maps, list(range(NCORES)))

    full = np.zeros((sched["nrows"], D_OUT), dtype=np.float32)
    for c in range(NCORES):
        oc = res.results[c]["out"]
        for j in range(sched["seg_cap"]):
            g = sched["out_map"][c, j]
            if g >= 0:
                full[g] = oc[j]
    return full
